# revision 52
# baseline (speedup 1.0000x reference)
"""Trainium2 Bass kernel for nn_Conv2dKan (KAN-style 3x3 conv, 64->128 ch).

Math: out[b,o,l] = sum_k silu(u)*w_b + sum_{n,k} H_n(u)*(c*w_s), with u =
unfold(x) (3x3, pad 1). Linear in the basis functions, so the Hermite basis
H_0..H_7 is re-expressed in the monomial basis {u, u^2, ..., u^7} with the
basis change folded into the weights on the host; silu itself is folded in
as a degree-7 least-squares polynomial fit over the actual input values.
Constant terms are a per-o bias added on the host after gather.

Device work per core (one batch item): the [x|x] input tile's upper half is
squared IN PLACE so it becomes implicit-GEMM chunk A = [x|x^2] with no
copies; chunks B=[x^3|x^4] and C=[x^5|x^6] come from a short ACT/DVE/Pool
chain; chunk D pairs the lonely 7th plane with its own column-shifted copy
[x^7 | x^7>>1col], which lets one matmul pass cover two filter taps - D
needs only 6 passes for its 9 taps (33 passes total instead of 36).
All matmuls are K=128 fp32r x fp32r, PSUM-accumulated per row tile (9+9+9+6
passes), staggered DVE evacuation + DMA out.  Input DMAs are fine-sliced
across queues so the first row tile and chunk-A weights land first.

Sharding: batch 8 -> one image per NeuronCore, fully data parallel.
"""

import sys

if "/opt/trn_rl_repo" not in sys.path:
    sys.path.insert(0, "/opt/trn_rl_repo")

import numpy as np

import concourse.bacc as bacc
import concourse.bass as bass
import concourse.tile as tile
from concourse import mybir
from concourse.bass_utils import run_bass_kernel_spmd

# Problem constants (hardcoded per harness contract).
B = 8
C_IN = 64
C_OUT = 128
K = 3
N_BASIS = 8
H = W = 48
HP = WP = H + 2  # padded image
L = H * W
PADN = HP * WP  # 2500
NTAPS = K * K
ROW_TILES = (10, 10, 10, 10, 8)
N_WARM = 15
ND = 6  # chunk-D passes (tap pairs)
OUT_SCALE = 64.0  # device output scale (fp16 overflow headroom)

_CACHE = {}


def _build_program():
    nc = bacc.Bacc("TRN2", target_bir_lowering=False, debug=False, num_devices=1)
    f32 = mybir.dt.float32
    f32r = mybir.dt.float32r
    ACT = mybir.ActivationFunctionType

    f16 = mybir.dt.float16
    xl_d = nc.dram_tensor("xl", [64, PADN], f16, kind="ExternalInput").ap()
    xu_d = nc.dram_tensor("xu", [64, PADN], f16, kind="ExternalInput").ap()
    wa_d = nc.dram_tensor("wa", [128, NTAPS * 128], f16, kind="ExternalInput").ap()
    w_d = nc.dram_tensor("w", [128, (2 * NTAPS + ND) * 128], f16, kind="ExternalInput").ap()
    o_d = nc.dram_tensor("out", [C_OUT, L], f16, kind="ExternalOutput").ap()

    XS = (625, 1250, 1875, PADN)
    CS = (0, 834, 1667, PADN)  # slice bounds for elementwise ops

    with tile.TileContext(nc) as tc:
        with (
            tc.tile_pool(name="big", bufs=1) as wpool,
            tc.tile_pool(name="outs", bufs=3) as opool,
            tc.tile_pool(name="psum", bufs=1, space="PSUM") as ppool,
        ):
            x_sb = wpool.tile([128, PADN], f16, tag="xx")        # A = [x|x^2/2]
            xu = wpool.tile([128, PADN], f16, tag="xu")          # fp16 x at partitions 64-127
            t2 = wpool.tile([128, PADN], f16, tag="t2")          # [x^2/2 | -]
            bt = wpool.tile([128, PADN], f16, tag="bt")          # B = [x^3/2|x^4/4]
            ct = wpool.tile([128, PADN], f16, tag="ct")          # C = [x^5/4|x^6/8]
            dt_ = wpool.tile([128, PADN], f16, tag="dt")         # D = [x^7/8|x^7/8>>1]
            wa16 = wpool.tile([128, NTAPS * 128], f16, tag="wa16")
            w16 = wpool.tile([128, (2 * NTAPS + ND) * 128], f16, tag="w16")
            warm = wpool.tile([128, 256], f16, tag="warm")

            LO = slice(0, 64)
            UP = slice(64, 128)

            # ---- input DMAs (all fp16 on the wire; fine-sliced) ----
            # x lower pieces on sync, upper pieces on gpsimd; chunk A's
            # upper comes from Square(xu) directly (ACT converts fp16).
            XQ5 = (0, 313, 625, 1250, 1875, PADN)
            nc.sync.dma_start(out=xu[64:128, 0:313], in_=xu_d[:, 0:313])
            nc.sync.dma_start(out=x_sb[0:64, 0:313], in_=xl_d[:, 0:313])
            nc.sync.dma_start(out=xu[64:128, 313:625], in_=xu_d[:, 313:625])
            for q in range(1, 5):
                nc.sync.dma_start(
                    out=x_sb[0:64, XQ5[q] : XQ5[q + 1]], in_=xl_d[:, XQ5[q] : XQ5[q + 1]]
                )
            nc.gpsimd.memset(dt_[UP, PADN - 1 : PADN], 0.0)
            for q in range(2, 5):
                nc.gpsimd.dma_start(
                    out=xu[64:128, XQ5[q] : XQ5[q + 1]], in_=xu_d[:, XQ5[q] : XQ5[q + 1]]
                )
            # chunk-A weights in 3-tap pieces on scalar (these gate mm0)
            for p in range(3):
                nc.scalar.dma_start(
                    out=wa16[:, p * 384 : (p + 1) * 384], in_=wa_d[:, p * 384 : (p + 1) * 384]
                )
            # chunks B, C, D: needed late (t=21/30/39us), so keep their
            # 1.1MB out of the critical DMA window. The tile scheduler
            # hoists dependency-free DMAs, so manufacture a WAR hazard: a
            # 1-element mul reads the w16 head AND the last xu piece; the
            # w16 DMAs (written after it) must then wait for xu to land.
            scr = wpool.tile([128, 8], f32, tag="scr")
            w16f = w16.bitcast(f32)
            nc.gpsimd.memset(w16f[64:65, 0:1], 0.0)
            nc.gpsimd.tensor_mul(
                scr[64:65, 0:1], w16f[64:65, 0:1], xu.bitcast(f32)[64:65, 1200:1201]
            )
            WB = NTAPS * 128
            HWB = WB // 2
            for p in range(2):
                nc.sync.dma_start(
                    out=w16[:, p * WB : (p + 1) * WB], in_=w_d[:, p * WB : (p + 1) * WB]
                )
            nc.sync.dma_start(out=w16[:, 2 * WB :], in_=w_d[:, 2 * WB :])

            # DVE: warm memset (warmup starts early); no casts needed,
            # the fp16 wire data feeds the PE directly
            nc.vector.memset(warm.bitcast(f32)[:], 0.0)

            # ---- PE pre-warm while DMAs land (HAM/pstate ramp) ----
            warm_ps = ppool.tile([128, 256], f32, tag="warm_ps")
            for _ in range(N_WARM):
                nc.tensor.matmul(warm_ps[:], warm[:, 0:128], warm[:], start=True, stop=True)

            # ---- feature planes ----
            # chunk-A upper = Square(xu), sliced to follow the x pieces so
            # chunk A's first row tile is ready as soon as possible
            RS2 = 0.7071067811865476  # 1/sqrt(2): Square(x*RS2) = x^2/2
            for q in range(5):
                xs_ = slice(XQ5[q], XQ5[q + 1])
                nc.scalar.activation(x_sb[UP, xs_], xu[UP, xs_], ACT.Square, scale=RS2)
            # lower: t2 = x^2/2; odd powers by repeated multiply with t2
            for b in range(3):
                cs = slice(CS[b], CS[b + 1])
                nc.scalar.activation(t2[LO, cs], x_sb[LO, cs], ACT.Square, scale=RS2)
                nc.vector.tensor_mul(bt[LO, cs], t2[LO, cs], x_sb[LO, cs])       # x^3/2
            for b in range(3):
                cs = slice(CS[b], CS[b + 1])
                nc.scalar.activation(bt[UP, cs], x_sb[UP, cs], ACT.Square)       # x^4/4
                nc.gpsimd.tensor_mul(ct[LO, cs], bt[LO, cs], t2[LO, cs])         # x^5/4
            for b in range(3):
                cs = slice(CS[b], CS[b + 1])
                nc.vector.tensor_mul(ct[UP, cs], bt[UP, cs], x_sb[UP, cs])       # x^6/8
                nc.gpsimd.tensor_mul(dt_[LO, cs], ct[LO, cs], t2[LO, cs])        # x^7/8
            # D upper = x^7/8 shifted left one col: x^6/8[c+1] * x[c+1]
            for b in range(3):
                c0, c1_ = CS[b], CS[b + 1]
                if c1_ == PADN:
                    c1_ = PADN - 1  # last col memset to 0 on gpsimd above
                nc.vector.tensor_mul(
                    dt_[UP, c0:c1_], ct[UP, c0 + 1 : c1_ + 1], xu[UP, c0 + 1 : c1_ + 1]
                )

            # ---- implicit GEMM: chunk-outer, tile-mid, pass-inner ----
            ims = [t.rearrange("c (h w) -> c h w", h=HP) for t in (x_sb, bt, ct, dt_)]
            psums = []
            h0s = []
            h0 = 0
            for it, R in enumerate(ROW_TILES):
                psums.append(ppool.tile([128, R * W], f32, name=f"ps{h0}", tag=f"ps{it}"))
                h0s.append(h0)
                h0 += R
            out_rings = (nc.sync, nc.gpsimd, nc.sync, nc.gpsimd)

            # chunks A, B, C: 9 taps each
            for j in range(3):
                im = ims[j]
                for it, R in enumerate(ROW_TILES):
                    h0 = h0s[it]
                    for t9 in range(NTAPS):
                        dh, dw = t9 // K - 1, t9 % K - 1
                        r0 = h0 + dh + 1
                        lhsT = (
                            wa16[:, t9 * 128 : (t9 + 1) * 128]
                            if j == 0
                            else w16[:, ((j - 1) * NTAPS + t9) * 128 : ((j - 1) * NTAPS + t9 + 1) * 128]
                        )
                        nc.tensor.matmul(
                            psums[it][:],
                            lhsT,
                            im[:, r0 : r0 + R, dw + 1 : dw + 1 + W],
                            start=(j == 0 and t9 == 0),
                            stop=False,
                        )
            # chunk D: 6 passes (dh x col-offset o in {0,1}); lower covers
            # tap (dh,o-1), shifted upper covers tap (dh,o)
            imd = ims[3]
            for it, R in enumerate(ROW_TILES):
                h0 = h0s[it]
                for p in range(ND):
                    dh, o = p // 2 - 1, p % 2
                    r0 = h0 + dh + 1
                    nc.tensor.matmul(
                        psums[it][:],
                        w16[:, (2 * NTAPS + p) * 128 : (2 * NTAPS + p + 1) * 128],
                        imd[:, r0 : r0 + R, o : o + W],
                        start=False,
                        stop=(p == ND - 1),
                    )
                # staggered evacuation: DVE PSUM->SBUF, then DMA out
                o_sb = opool.tile([C_OUT, R * W], f16, tag="osb")
                if it < len(ROW_TILES) - 1:
                    nc.vector.tensor_copy(o_sb[:], psums[it][:])
                    out_rings[it].dma_start(out=o_d[:, h0 * W : (h0 + R) * W], in_=o_sb[:])
                else:
                    # last tile: halve so the final DMA starts sooner; both
                    # halves go out the fast sync queue (gpsimd's software
                    # queue is ~2x slower)
                    hn = R * W // 2
                    for hh, eng in ((0, nc.sync), (1, nc.sync)):
                        nc.vector.tensor_copy(
                            o_sb[:, hh * hn : (hh + 1) * hn],
                            psums[it][:, hh * hn : (hh + 1) * hn],
                        )
                        eng.dma_start(
                            out=o_d[:, h0 * W + hh * hn : h0 * W + (hh + 1) * hn],
                            in_=o_sb[:, hh * hn : (hh + 1) * hn],
                        )

    nc.compile()
    return nc


def _host_prep(x, w_b, w_s, c):
    """Fold Hermite->monomial basis change, w_s, and a degree-7 polynomial
    fit of silu into the weights (fp64 host math)."""
    wb = w_b[..., 0].astype(np.float64)          # (O, 576)
    cw = (c[..., 0] * w_s[None, ..., 0]).astype(np.float64)  # (N, O, 576)

    # monomial weights for planes u^1..u^7 (+ constant -> bias)
    wm = np.zeros((8, C_OUT, C_IN * NTAPS), np.float64)
    wm[1] = 2 * cw[1] - 12 * cw[3] + 120 * cw[5] - 1680 * cw[7]
    wm[2] = 2 * cw[2] - 48 * cw[4] + 720 * cw[6]
    wm[3] = 8 * cw[3] - 160 * cw[5] + 3360 * cw[7]
    wm[4] = 16 * cw[4] - 480 * cw[6]
    wm[5] = 32 * cw[5] - 1344 * cw[7]
    wm[6] = 64 * cw[6]
    wm[7] = 128 * cw[7]
    bias = (cw[0] - 2 * cw[2] + 12 * cw[4] - 120 * cw[6]).sum(axis=1)  # (O,)

    # degree-7 LS fit of silu over the actual input values (+ Chebyshev
    # nodes over the input range for tail control), folded into wm/bias
    xs = np.asarray(x, np.float64).ravel()
    m = np.abs(xs).max() * 1.02
    nodes = m * np.cos(np.pi * (np.arange(2000) + 0.5) / 2000)
    fitx = np.concatenate([xs[::37], nodes, nodes, nodes])
    A = np.vander(fitx, 8, increasing=True)
    coef, *_ = np.linalg.lstsq(A, fitx / (1 + np.exp(-fitx)), rcond=None)
    for f in range(1, 8):
        wm[f] += coef[f] * wb
    bias = bias + coef[0] * wb.sum(axis=1)

    # scale so the fp16 device output can't overflow; undone on the host
    wm *= 1.0 / OUT_SCALE

    cidx = np.arange(C_IN)

    PSC = {1: 1.0, 2: 2.0, 3: 2.0, 4: 4.0, 5: 4.0, 6: 8.0, 7: 8.0}

    def tapw(f, t):
        # [64, 128] weight block: plane u^f/PSC[f], tap t
        return (wm[f][:, cidx * NTAPS + t].T * PSC[f]).astype(np.float32)

    # chunk A = [x | x^2]: 9 taps
    wa = np.zeros((128, NTAPS, C_OUT), np.float32)
    for t in range(NTAPS):
        wa[0:64, t] = tapw(1, t)
        wa[64:128, t] = tapw(2, t)
    # chunks B = [x^3|x^4], C = [x^5|x^6]: 9 taps each
    wl = np.zeros((128, 2 * NTAPS + ND, C_OUT), np.float32)
    for t in range(NTAPS):
        wl[0:64, t] = tapw(3, t)
        wl[64:128, t] = tapw(4, t)
        wl[0:64, NTAPS + t] = tapw(5, t)
        wl[64:128, NTAPS + t] = tapw(6, t)
    # chunk D = [x^7 | x^7>>1]: 6 passes (dh, o); lower = tap (dh, o-1),
    # upper = tap (dh, 1) on o==1 passes only
    for p in range(ND):
        dh, o = p // 2 - 1, p % 2
        t_lo = (dh + 1) * K + (o - 1 + 1)
        wl[0:64, 2 * NTAPS + p] = tapw(7, t_lo)
        if o == 1:
            wl[64:128, 2 * NTAPS + p] = tapw(7, (dh + 1) * K + 2)
    return (
        wa.reshape(128, NTAPS * 128).astype(np.float16),
        wl.reshape(128, (2 * NTAPS + ND) * 128).astype(np.float16),
        bias.astype(np.float32),
    )


def _prep_in_maps(x, w_b, w_s, c):
    wa, wl, bias = _host_prep(x, w_b, w_s, c)
    xi = np.asarray(x, np.float16)
    xp = np.zeros((B, C_IN, HP, WP), np.float16)
    xp[:, :, 1 : 1 + H, 1 : 1 + W] = xi
    xp = xp.reshape(B, C_IN, PADN)
    in_maps = []
    for i in range(B):
        in_maps.append({"xl": xp[i], "xu": xp[i], "wa": wa, "w": wl})
    return in_maps, bias


def kernel(x, w_b, w_s, c):
    if "nc" not in _CACHE:
        _CACHE["nc"] = _build_program()
    nc = _CACHE["nc"]

    in_maps, bias = _prep_in_maps(x, w_b, w_s, c)
    res = run_bass_kernel_spmd(nc, in_maps, core_ids=list(range(B)))
    out = np.stack(
        [res.results[i]["out"].astype(np.float32) for i in range(B)], axis=0
    )
    out *= OUT_SCALE
    out += bias[None, :, None]
    return out.reshape(B, C_OUT, H, W)


# revision 53
# speedup vs baseline: 1.0159x; 1.0159x over previous
"""Trainium2 Bass kernel for nn_Conv2dKan (KAN-style 3x3 conv, 64->128 ch).

Math: out[b,o,l] = sum_k silu(u)*w_b + sum_{n,k} H_n(u)*(c*w_s), with u =
unfold(x) (3x3, pad 1). Linear in the basis functions, so the Hermite basis
H_0..H_7 is re-expressed in the monomial basis {u, u^2, ..., u^7} with the
basis change folded into the weights on the host; silu itself is folded in
as a degree-7 least-squares polynomial fit over the actual input values.
Constant terms are a per-o bias added on the host after gather.

Device work per core (one batch item): the [x|x] input tile's upper half is
squared IN PLACE so it becomes implicit-GEMM chunk A = [x|x^2] with no
copies; chunks B=[x^3|x^4] and C=[x^5|x^6] come from a short ACT/DVE/Pool
chain; chunk D pairs the lonely 7th plane with its own column-shifted copy
[x^7 | x^7>>1col], which lets one matmul pass cover two filter taps - D
needs only 6 passes for its 9 taps (33 passes total instead of 36).
All matmuls are K=128 fp32r x fp32r, PSUM-accumulated per row tile (9+9+9+6
passes), staggered DVE evacuation + DMA out.  Input DMAs are fine-sliced
across queues so the first row tile and chunk-A weights land first.

Sharding: batch 8 -> one image per NeuronCore, fully data parallel.
"""

import sys

if "/opt/trn_rl_repo" not in sys.path:
    sys.path.insert(0, "/opt/trn_rl_repo")

import numpy as np

import concourse.bacc as bacc
import concourse.bass as bass
import concourse.tile as tile
from concourse import mybir
from concourse.bass_utils import run_bass_kernel_spmd

# Problem constants (hardcoded per harness contract).
B = 8
C_IN = 64
C_OUT = 128
K = 3
N_BASIS = 8
H = W = 48
HP = WP = H + 2  # padded image
L = H * W
PADN = HP * WP  # 2500
NTAPS = K * K
ROW_TILES = (10, 10, 10, 10, 8)
N_WARM = 15
ND = 6  # chunk-D passes (tap pairs)
OUT_SCALE = 64.0  # device output scale (fp16 overflow headroom)

_CACHE = {}


def _build_program():
    nc = bacc.Bacc("TRN2", target_bir_lowering=False, debug=False, num_devices=1)
    f32 = mybir.dt.float32
    f32r = mybir.dt.float32r
    ACT = mybir.ActivationFunctionType

    f16 = mybir.dt.float16
    xl_d = nc.dram_tensor("xl", [64, PADN], f16, kind="ExternalInput").ap()
    xu_d = nc.dram_tensor("xu", [64, PADN], f16, kind="ExternalInput").ap()
    wa_d = nc.dram_tensor("wa", [128, NTAPS * 128], f16, kind="ExternalInput").ap()
    w_d = nc.dram_tensor("w", [128, (2 * NTAPS + ND) * 128], f16, kind="ExternalInput").ap()
    o_d = nc.dram_tensor("out", [C_OUT, L], f16, kind="ExternalOutput").ap()

    XS = (625, 1250, 1875, PADN)
    CS = (0, 834, 1667, PADN)  # slice bounds for elementwise ops

    with tile.TileContext(nc) as tc:
        with (
            tc.tile_pool(name="big", bufs=1) as wpool,
            tc.tile_pool(name="outs", bufs=3) as opool,
            tc.tile_pool(name="psum", bufs=1, space="PSUM") as ppool,
        ):
            x_sb = wpool.tile([128, PADN], f16, tag="xx")        # A = [x|x^2/2]
            xu = wpool.tile([128, PADN], f16, tag="xu")          # fp16 x at partitions 64-127
            t2 = wpool.tile([128, PADN], f16, tag="t2")          # [x^2/2 | -]
            bt = wpool.tile([128, PADN], f16, tag="bt")          # B = [x^3/2|x^4/4]
            ct = wpool.tile([128, PADN], f16, tag="ct")          # C = [x^5/4|x^6/8]
            dt_ = wpool.tile([128, PADN], f16, tag="dt")         # D = [x^7/8|x^7/8>>1]
            wa16 = wpool.tile([128, NTAPS * 128], f16, tag="wa16")
            w16 = wpool.tile([128, (2 * NTAPS + ND) * 128], f16, tag="w16")
            warm = wpool.tile([128, 256], f16, tag="warm")

            LO = slice(0, 64)
            UP = slice(64, 128)

            # ---- input DMAs (all fp16 on the wire; fine-sliced) ----
            # x lower pieces on sync, upper pieces on gpsimd; chunk A's
            # upper comes from Square(xu) directly (ACT converts fp16).
            XQ5 = (0, 313, 625, 1250, 1875, PADN)
            for q in range(5):
                nc.sync.dma_start(
                    out=x_sb[0:64, XQ5[q] : XQ5[q + 1]], in_=xl_d[:, XQ5[q] : XQ5[q + 1]]
                )
            nc.gpsimd.dma_start(out=xu[64:128, 0:313], in_=xu_d[:, 0:313])
            nc.gpsimd.memset(dt_[UP, PADN - 1 : PADN], 0.0)
            for q in range(1, 5):
                nc.gpsimd.dma_start(
                    out=xu[64:128, XQ5[q] : XQ5[q + 1]], in_=xu_d[:, XQ5[q] : XQ5[q + 1]]
                )
            # chunk-A weights in 3-tap pieces on scalar (these gate mm0)
            for p in range(3):
                nc.scalar.dma_start(
                    out=wa16[:, p * 384 : (p + 1) * 384], in_=wa_d[:, p * 384 : (p + 1) * 384]
                )
            # chunks B, C, D: needed late (t=21/30/39us), so keep their
            # 1.1MB out of the critical DMA window. The tile scheduler
            # hoists dependency-free DMAs, so manufacture a WAR hazard: a
            # 1-element mul reads the w16 head AND the last xu piece; the
            # w16 DMAs (written after it) must then wait for xu to land.
            scr = wpool.tile([128, 8], f32, tag="scr")
            w16f = w16.bitcast(f32)
            nc.gpsimd.memset(w16f[64:65, 0:1], 0.0)
            nc.gpsimd.tensor_mul(
                scr[64:65, 0:1], w16f[64:65, 0:1], xu.bitcast(f32)[64:65, 1200:1201]
            )
            WB = NTAPS * 128
            HWB = WB // 2
            for p in range(2):
                nc.sync.dma_start(
                    out=w16[:, p * WB : (p + 1) * WB], in_=w_d[:, p * WB : (p + 1) * WB]
                )
            nc.sync.dma_start(out=w16[:, 2 * WB :], in_=w_d[:, 2 * WB :])

            # DVE: warm memset (warmup starts early); no casts needed,
            # the fp16 wire data feeds the PE directly
            nc.vector.memset(warm.bitcast(f32)[:], 0.0)

            # ---- PE pre-warm while DMAs land (HAM/pstate ramp) ----
            warm_ps = ppool.tile([128, 256], f32, tag="warm_ps")
            for _ in range(N_WARM):
                nc.tensor.matmul(warm_ps[:], warm[:, 0:128], warm[:], start=True, stop=True)

            # ---- feature planes ----
            # chunk-A upper = Square(xu), sliced to follow the x pieces so
            # chunk A's first row tile is ready as soon as possible
            RS2 = 0.7071067811865476  # 1/sqrt(2): Square(x*RS2) = x^2/2
            for q in range(5):
                xs_ = slice(XQ5[q], XQ5[q + 1])
                nc.scalar.activation(x_sb[UP, xs_], xu[UP, xs_], ACT.Square, scale=RS2)
            # lower: t2 = x^2/2; odd powers by repeated multiply with t2
            for b in range(3):
                cs = slice(CS[b], CS[b + 1])
                nc.scalar.activation(t2[LO, cs], x_sb[LO, cs], ACT.Square, scale=RS2)
                nc.vector.tensor_mul(bt[LO, cs], t2[LO, cs], x_sb[LO, cs])       # x^3/2
            for b in range(3):
                cs = slice(CS[b], CS[b + 1])
                nc.scalar.activation(bt[UP, cs], x_sb[UP, cs], ACT.Square)       # x^4/4
                nc.gpsimd.tensor_mul(ct[LO, cs], bt[LO, cs], t2[LO, cs])         # x^5/4
            for b in range(3):
                cs = slice(CS[b], CS[b + 1])
                nc.vector.tensor_mul(ct[UP, cs], bt[UP, cs], x_sb[UP, cs])       # x^6/8
                nc.gpsimd.tensor_mul(dt_[LO, cs], ct[LO, cs], t2[LO, cs])        # x^7/8
            # D upper = x^7/8 shifted left one col: x^6/8[c+1] * x[c+1]
            for b in range(3):
                c0, c1_ = CS[b], CS[b + 1]
                if c1_ == PADN:
                    c1_ = PADN - 1  # last col memset to 0 on gpsimd above
                nc.vector.tensor_mul(
                    dt_[UP, c0:c1_], ct[UP, c0 + 1 : c1_ + 1], xu[UP, c0 + 1 : c1_ + 1]
                )

            # ---- implicit GEMM: chunk-outer, tile-mid, pass-inner ----
            ims = [t.rearrange("c (h w) -> c h w", h=HP) for t in (x_sb, bt, ct, dt_)]
            psums = []
            h0s = []
            h0 = 0
            for it, R in enumerate(ROW_TILES):
                psums.append(ppool.tile([128, R * W], f32, name=f"ps{h0}", tag=f"ps{it}"))
                h0s.append(h0)
                h0 += R
            out_rings = (nc.sync, nc.gpsimd, nc.sync, nc.gpsimd)

            # chunks A, B, C: 9 taps each
            for j in range(3):
                im = ims[j]
                for it, R in enumerate(ROW_TILES):
                    h0 = h0s[it]
                    for t9 in range(NTAPS):
                        dh, dw = t9 // K - 1, t9 % K - 1
                        r0 = h0 + dh + 1
                        lhsT = (
                            wa16[:, t9 * 128 : (t9 + 1) * 128]
                            if j == 0
                            else w16[:, ((j - 1) * NTAPS + t9) * 128 : ((j - 1) * NTAPS + t9 + 1) * 128]
                        )
                        nc.tensor.matmul(
                            psums[it][:],
                            lhsT,
                            im[:, r0 : r0 + R, dw + 1 : dw + 1 + W],
                            start=(j == 0 and t9 == 0),
                            stop=False,
                        )
            # chunk D: 6 passes (dh x col-offset o in {0,1}); lower covers
            # tap (dh,o-1), shifted upper covers tap (dh,o)
            imd = ims[3]
            for it, R in enumerate(ROW_TILES):
                h0 = h0s[it]
                for p in range(ND):
                    dh, o = p // 2 - 1, p % 2
                    r0 = h0 + dh + 1
                    nc.tensor.matmul(
                        psums[it][:],
                        w16[:, (2 * NTAPS + p) * 128 : (2 * NTAPS + p + 1) * 128],
                        imd[:, r0 : r0 + R, o : o + W],
                        start=False,
                        stop=(p == ND - 1),
                    )
                # staggered evacuation: DVE PSUM->SBUF, then DMA out
                o_sb = opool.tile([C_OUT, R * W], f16, tag="osb")
                if it < len(ROW_TILES) - 1:
                    nc.vector.tensor_copy(o_sb[:], psums[it][:])
                    out_rings[it].dma_start(out=o_d[:, h0 * W : (h0 + R) * W], in_=o_sb[:])
                else:
                    # last tile: halve so the final DMA starts sooner
                    hn = R * W // 2
                    for hh, eng in ((0, nc.sync), (1, nc.sync)):
                        nc.vector.tensor_copy(
                            o_sb[:, hh * hn : (hh + 1) * hn],
                            psums[it][:, hh * hn : (hh + 1) * hn],
                        )
                        eng.dma_start(
                            out=o_d[:, h0 * W + hh * hn : h0 * W + (hh + 1) * hn],
                            in_=o_sb[:, hh * hn : (hh + 1) * hn],
                        )

    nc.compile()
    return nc


def _host_prep(x, w_b, w_s, c):
    """Fold Hermite->monomial basis change, w_s, and a degree-7 polynomial
    fit of silu into the weights (fp64 host math)."""
    wb = w_b[..., 0].astype(np.float64)          # (O, 576)
    cw = (c[..., 0] * w_s[None, ..., 0]).astype(np.float64)  # (N, O, 576)

    # monomial weights for planes u^1..u^7 (+ constant -> bias)
    wm = np.zeros((8, C_OUT, C_IN * NTAPS), np.float64)
    wm[1] = 2 * cw[1] - 12 * cw[3] + 120 * cw[5] - 1680 * cw[7]
    wm[2] = 2 * cw[2] - 48 * cw[4] + 720 * cw[6]
    wm[3] = 8 * cw[3] - 160 * cw[5] + 3360 * cw[7]
    wm[4] = 16 * cw[4] - 480 * cw[6]
    wm[5] = 32 * cw[5] - 1344 * cw[7]
    wm[6] = 64 * cw[6]
    wm[7] = 128 * cw[7]
    bias = (cw[0] - 2 * cw[2] + 12 * cw[4] - 120 * cw[6]).sum(axis=1)  # (O,)

    # degree-7 LS fit of silu over the actual input values (+ Chebyshev
    # nodes over the input range for tail control), folded into wm/bias
    xs = np.asarray(x, np.float64).ravel()
    m = np.abs(xs).max() * 1.02
    nodes = m * np.cos(np.pi * (np.arange(2000) + 0.5) / 2000)
    fitx = np.concatenate([xs[::37], nodes, nodes, nodes])
    A = np.vander(fitx, 8, increasing=True)
    coef, *_ = np.linalg.lstsq(A, fitx / (1 + np.exp(-fitx)), rcond=None)
    for f in range(1, 8):
        wm[f] += coef[f] * wb
    bias = bias + coef[0] * wb.sum(axis=1)

    # scale so the fp16 device output can't overflow; undone on the host
    wm *= 1.0 / OUT_SCALE

    cidx = np.arange(C_IN)

    PSC = {1: 1.0, 2: 2.0, 3: 2.0, 4: 4.0, 5: 4.0, 6: 8.0, 7: 8.0}

    def tapw(f, t):
        # [64, 128] weight block: plane u^f/PSC[f], tap t
        return (wm[f][:, cidx * NTAPS + t].T * PSC[f]).astype(np.float32)

    # chunk A = [x | x^2]: 9 taps
    wa = np.zeros((128, NTAPS, C_OUT), np.float32)
    for t in range(NTAPS):
        wa[0:64, t] = tapw(1, t)
        wa[64:128, t] = tapw(2, t)
    # chunks B = [x^3|x^4], C = [x^5|x^6]: 9 taps each
    wl = np.zeros((128, 2 * NTAPS + ND, C_OUT), np.float32)
    for t in range(NTAPS):
        wl[0:64, t] = tapw(3, t)
        wl[64:128, t] = tapw(4, t)
        wl[0:64, NTAPS + t] = tapw(5, t)
        wl[64:128, NTAPS + t] = tapw(6, t)
    # chunk D = [x^7 | x^7>>1]: 6 passes (dh, o); lower = tap (dh, o-1),
    # upper = tap (dh, 1) on o==1 passes only
    for p in range(ND):
        dh, o = p // 2 - 1, p % 2
        t_lo = (dh + 1) * K + (o - 1 + 1)
        wl[0:64, 2 * NTAPS + p] = tapw(7, t_lo)
        if o == 1:
            wl[64:128, 2 * NTAPS + p] = tapw(7, (dh + 1) * K + 2)
    return (
        wa.reshape(128, NTAPS * 128).astype(np.float16),
        wl.reshape(128, (2 * NTAPS + ND) * 128).astype(np.float16),
        bias.astype(np.float32),
    )


def _prep_in_maps(x, w_b, w_s, c):
    wa, wl, bias = _host_prep(x, w_b, w_s, c)
    xi = np.asarray(x, np.float16)
    xp = np.zeros((B, C_IN, HP, WP), np.float16)
    xp[:, :, 1 : 1 + H, 1 : 1 + W] = xi
    xp = xp.reshape(B, C_IN, PADN)
    in_maps = []
    for i in range(B):
        in_maps.append({"xl": xp[i], "xu": xp[i], "wa": wa, "w": wl})
    return in_maps, bias


def kernel(x, w_b, w_s, c):
    if "nc" not in _CACHE:
        _CACHE["nc"] = _build_program()
    nc = _CACHE["nc"]

    in_maps, bias = _prep_in_maps(x, w_b, w_s, c)
    res = run_bass_kernel_spmd(nc, in_maps, core_ids=list(range(B)))
    out = np.stack(
        [res.results[i]["out"].astype(np.float32) for i in range(B)], axis=0
    )
    out *= OUT_SCALE
    out += bias[None, :, None]
    return out.reshape(B, C_OUT, H, W)


# revision 54
# speedup vs baseline: 1.0567x; 1.0401x over previous
"""Trainium2 Bass kernel for nn_Conv2dKan (KAN-style 3x3 conv, 64->128 ch).

Math: out[b,o,l] = sum_k silu(u)*w_b + sum_{n,k} H_n(u)*(c*w_s), with u =
unfold(x) (3x3, pad 1). Linear in the basis functions, so the Hermite basis
H_0..H_7 is re-expressed in the monomial basis {u, u^2, ..., u^7} with the
basis change folded into the weights on the host; silu itself is folded in
as a degree-7 least-squares polynomial fit over the actual input values.
Constant terms are a per-o bias added on the host after gather.

Device work per core (one batch item): the [x|x] input tile's upper half is
squared IN PLACE so it becomes implicit-GEMM chunk A = [x|x^2] with no
copies; chunks B=[x^3|x^4] and C=[x^5|x^6] come from a short ACT/DVE/Pool
chain; chunk D pairs the lonely 7th plane with its own column-shifted copy
[x^7 | x^7>>1col], which lets one matmul pass cover two filter taps - D
needs only 6 passes for its 9 taps (33 passes total instead of 36).
All matmuls are K=128 fp32r x fp32r, PSUM-accumulated per row tile (9+9+9+6
passes), staggered DVE evacuation + DMA out.  Input DMAs are fine-sliced
across queues so the first row tile and chunk-A weights land first.

Sharding: batch 8 -> one image per NeuronCore, fully data parallel.
"""

import sys

if "/opt/trn_rl_repo" not in sys.path:
    sys.path.insert(0, "/opt/trn_rl_repo")

import numpy as np

import concourse.bacc as bacc
import concourse.bass as bass
import concourse.tile as tile
from concourse import mybir
from concourse.bass_utils import run_bass_kernel_spmd

# Problem constants (hardcoded per harness contract).
B = 8
C_IN = 64
C_OUT = 128
K = 3
N_BASIS = 8
H = W = 48
HP = WP = H + 2  # padded image
L = H * W
PADN = HP * WP  # 2500
NTAPS = K * K
ROW_TILES = (10, 10, 10, 10, 8)
N_WARM = 18
ND = 6  # chunk-D passes (tap pairs)
OUT_SCALE = 64.0  # device output scale (fp16 overflow headroom)

_CACHE = {}


def _build_program():
    nc = bacc.Bacc("TRN2", target_bir_lowering=False, debug=False, num_devices=1)
    f32 = mybir.dt.float32
    f32r = mybir.dt.float32r
    ACT = mybir.ActivationFunctionType

    f16 = mybir.dt.float16
    xl_d = nc.dram_tensor("xl", [64, PADN], f16, kind="ExternalInput").ap()
    xu_d = nc.dram_tensor("xu", [64, PADN], f16, kind="ExternalInput").ap()
    wa_d = nc.dram_tensor("wa", [128, NTAPS * 128], f16, kind="ExternalInput").ap()
    w_d = nc.dram_tensor("w", [128, (2 * NTAPS + ND) * 128], f16, kind="ExternalInput").ap()
    o_d = nc.dram_tensor("out", [C_OUT, L], f16, kind="ExternalOutput").ap()

    XS = (625, 1250, 1875, PADN)
    CS = (0, 834, 1667, PADN)  # slice bounds for elementwise ops

    with tile.TileContext(nc) as tc:
        with (
            tc.tile_pool(name="big", bufs=1) as wpool,
            tc.tile_pool(name="outs", bufs=3) as opool,
            tc.tile_pool(name="psum", bufs=1, space="PSUM") as ppool,
        ):
            x_sb = wpool.tile([128, PADN], f16, tag="xx")        # A = [x|x^2/2]
            xu = wpool.tile([128, PADN], f16, tag="xu")          # fp16 x at partitions 64-127
            t2 = wpool.tile([128, PADN], f16, tag="t2")          # [x^2/2 | -]
            bt = wpool.tile([128, PADN], f16, tag="bt")          # B = [x^3/2|x^4/4]
            ct = wpool.tile([128, PADN], f16, tag="ct")          # C = [x^5/4|x^6/8]
            dt_ = wpool.tile([128, PADN], f16, tag="dt")         # D = [x^7/8|x^7/8>>1]
            wa16 = wpool.tile([128, NTAPS * 128], f16, tag="wa16")
            w16 = wpool.tile([128, (2 * NTAPS + ND) * 128], f16, tag="w16")
            warm = wpool.tile([128, 256], f16, tag="warm")

            LO = slice(0, 64)
            UP = slice(64, 128)

            # ---- input DMAs (all fp16 on the wire; fine-sliced) ----
            # x lower pieces on sync, upper pieces on gpsimd; chunk A's
            # upper comes from Square(xu) directly (ACT converts fp16).
            XQ5 = (0, 313, 625, 1250, 1875, PADN)
            for q in range(5):
                nc.sync.dma_start(
                    out=x_sb[0:64, XQ5[q] : XQ5[q + 1]], in_=xl_d[:, XQ5[q] : XQ5[q + 1]]
                )
            nc.gpsimd.dma_start(out=xu[64:128, 0:313], in_=xu_d[:, 0:313])
            nc.gpsimd.memset(dt_[UP, PADN - 1 : PADN], 0.0)
            for q in range(1, 5):
                nc.gpsimd.dma_start(
                    out=xu[64:128, XQ5[q] : XQ5[q + 1]], in_=xu_d[:, XQ5[q] : XQ5[q + 1]]
                )
            # chunk-A weights in 3-tap pieces on scalar (these gate mm0)
            for p in range(3):
                nc.scalar.dma_start(
                    out=wa16[:, p * 384 : (p + 1) * 384], in_=wa_d[:, p * 384 : (p + 1) * 384]
                )
            # chunks B, C, D: needed late (t=21/30/39us), so keep their
            # 1.1MB out of the critical DMA window. The tile scheduler
            # hoists dependency-free DMAs, so manufacture a WAR hazard: a
            # 1-element mul reads the w16 head AND the last xu piece; the
            # w16 DMAs (written after it) must then wait for xu to land.
            scr = wpool.tile([128, 8], f32, tag="scr")
            w16f = w16.bitcast(f32)
            nc.gpsimd.memset(w16f[64:65, 0:1], 0.0)
            nc.gpsimd.tensor_mul(
                scr[64:65, 0:1], w16f[64:65, 0:1], xu.bitcast(f32)[64:65, 1200:1201]
            )
            WB = NTAPS * 128
            HWB = WB // 2
            for p in range(2):
                nc.sync.dma_start(
                    out=w16[:, p * WB : (p + 1) * WB], in_=w_d[:, p * WB : (p + 1) * WB]
                )
            nc.sync.dma_start(out=w16[:, 2 * WB :], in_=w_d[:, 2 * WB :])

            # DVE: warm memset (warmup starts early); no casts needed,
            # the fp16 wire data feeds the PE directly
            nc.vector.memset(warm.bitcast(f32)[:], 0.0)

            # ---- PE pre-warm while DMAs land (HAM/pstate ramp) ----
            warm_ps = ppool.tile([128, 256], f32, tag="warm_ps")
            for _ in range(N_WARM):
                nc.tensor.matmul(warm_ps[:], warm[:, 0:128], warm[:], start=True, stop=True)

            # ---- feature planes ----
            # chunk-A upper = Square(xu), sliced to follow the x pieces so
            # chunk A's first row tile is ready as soon as possible
            RS2 = 0.7071067811865476  # 1/sqrt(2): Square(x*RS2) = x^2/2
            for q in range(5):
                xs_ = slice(XQ5[q], XQ5[q + 1])
                nc.scalar.activation(x_sb[UP, xs_], xu[UP, xs_], ACT.Square, scale=RS2)
            # lower: t2 = x^2/2; odd powers by repeated multiply with t2
            for b in range(3):
                cs = slice(CS[b], CS[b + 1])
                nc.scalar.activation(t2[LO, cs], x_sb[LO, cs], ACT.Square, scale=RS2)
                nc.vector.tensor_mul(bt[LO, cs], t2[LO, cs], x_sb[LO, cs])       # x^3/2
            for b in range(3):
                cs = slice(CS[b], CS[b + 1])
                nc.scalar.activation(bt[UP, cs], x_sb[UP, cs], ACT.Square)       # x^4/4
                nc.gpsimd.tensor_mul(ct[LO, cs], bt[LO, cs], t2[LO, cs])         # x^5/4
            for b in range(3):
                cs = slice(CS[b], CS[b + 1])
                nc.vector.tensor_mul(ct[UP, cs], bt[UP, cs], x_sb[UP, cs])       # x^6/8
                nc.gpsimd.tensor_mul(dt_[LO, cs], ct[LO, cs], t2[LO, cs])        # x^7/8
            # D upper = x^7/8 shifted left one col: x^6/8[c+1] * x[c+1]
            for b in range(3):
                c0, c1_ = CS[b], CS[b + 1]
                if c1_ == PADN:
                    c1_ = PADN - 1  # last col memset to 0 on gpsimd above
                nc.vector.tensor_mul(
                    dt_[UP, c0:c1_], ct[UP, c0 + 1 : c1_ + 1], xu[UP, c0 + 1 : c1_ + 1]
                )

            # ---- implicit GEMM: chunk-outer, tile-mid, pass-inner ----
            ims = [t.rearrange("c (h w) -> c h w", h=HP) for t in (x_sb, bt, ct, dt_)]
            psums = []
            h0s = []
            h0 = 0
            for it, R in enumerate(ROW_TILES):
                psums.append(ppool.tile([128, R * W], f32, name=f"ps{h0}", tag=f"ps{it}"))
                h0s.append(h0)
                h0 += R
            out_rings = (nc.sync, nc.gpsimd, nc.sync, nc.gpsimd)

            # chunks A, B, C: 9 taps each
            for j in range(3):
                im = ims[j]
                for it, R in enumerate(ROW_TILES):
                    h0 = h0s[it]
                    for t9 in range(NTAPS):
                        dh, dw = t9 // K - 1, t9 % K - 1
                        r0 = h0 + dh + 1
                        lhsT = (
                            wa16[:, t9 * 128 : (t9 + 1) * 128]
                            if j == 0
                            else w16[:, ((j - 1) * NTAPS + t9) * 128 : ((j - 1) * NTAPS + t9 + 1) * 128]
                        )
                        nc.tensor.matmul(
                            psums[it][:],
                            lhsT,
                            im[:, r0 : r0 + R, dw + 1 : dw + 1 + W],
                            start=(j == 0 and t9 == 0),
                            stop=False,
                        )
            # chunk D: 6 passes (dh x col-offset o in {0,1}); lower covers
            # tap (dh,o-1), shifted upper covers tap (dh,o)
            imd = ims[3]
            for it, R in enumerate(ROW_TILES):
                h0 = h0s[it]
                for p in range(ND):
                    dh, o = p // 2 - 1, p % 2
                    r0 = h0 + dh + 1
                    nc.tensor.matmul(
                        psums[it][:],
                        w16[:, (2 * NTAPS + p) * 128 : (2 * NTAPS + p + 1) * 128],
                        imd[:, r0 : r0 + R, o : o + W],
                        start=False,
                        stop=(p == ND - 1),
                    )
                # staggered evacuation: DVE PSUM->SBUF, then DMA out
                o_sb = opool.tile([C_OUT, R * W], f16, tag="osb")
                if it < len(ROW_TILES) - 1:
                    nc.vector.tensor_copy(o_sb[:], psums[it][:])
                    out_rings[it].dma_start(out=o_d[:, h0 * W : (h0 + R) * W], in_=o_sb[:])
                else:
                    # last tile: halve so the final DMA starts sooner
                    hn = R * W // 2
                    for hh, eng in ((0, nc.sync), (1, nc.sync)):
                        nc.vector.tensor_copy(
                            o_sb[:, hh * hn : (hh + 1) * hn],
                            psums[it][:, hh * hn : (hh + 1) * hn],
                        )
                        eng.dma_start(
                            out=o_d[:, h0 * W + hh * hn : h0 * W + (hh + 1) * hn],
                            in_=o_sb[:, hh * hn : (hh + 1) * hn],
                        )

    nc.compile()
    return nc


def _host_prep(x, w_b, w_s, c):
    """Fold Hermite->monomial basis change, w_s, and a degree-7 polynomial
    fit of silu into the weights (fp64 host math)."""
    wb = w_b[..., 0].astype(np.float64)          # (O, 576)
    cw = (c[..., 0] * w_s[None, ..., 0]).astype(np.float64)  # (N, O, 576)

    # monomial weights for planes u^1..u^7 (+ constant -> bias)
    wm = np.zeros((8, C_OUT, C_IN * NTAPS), np.float64)
    wm[1] = 2 * cw[1] - 12 * cw[3] + 120 * cw[5] - 1680 * cw[7]
    wm[2] = 2 * cw[2] - 48 * cw[4] + 720 * cw[6]
    wm[3] = 8 * cw[3] - 160 * cw[5] + 3360 * cw[7]
    wm[4] = 16 * cw[4] - 480 * cw[6]
    wm[5] = 32 * cw[5] - 1344 * cw[7]
    wm[6] = 64 * cw[6]
    wm[7] = 128 * cw[7]
    bias = (cw[0] - 2 * cw[2] + 12 * cw[4] - 120 * cw[6]).sum(axis=1)  # (O,)

    # degree-7 LS fit of silu over the actual input values (+ Chebyshev
    # nodes over the input range for tail control), folded into wm/bias
    xs = np.asarray(x, np.float64).ravel()
    m = np.abs(xs).max() * 1.02
    nodes = m * np.cos(np.pi * (np.arange(2000) + 0.5) / 2000)
    fitx = np.concatenate([xs[::37], nodes, nodes, nodes])
    A = np.vander(fitx, 8, increasing=True)
    coef, *_ = np.linalg.lstsq(A, fitx / (1 + np.exp(-fitx)), rcond=None)
    for f in range(1, 8):
        wm[f] += coef[f] * wb
    bias = bias + coef[0] * wb.sum(axis=1)

    # scale so the fp16 device output can't overflow; undone on the host
    wm *= 1.0 / OUT_SCALE

    cidx = np.arange(C_IN)

    PSC = {1: 1.0, 2: 2.0, 3: 2.0, 4: 4.0, 5: 4.0, 6: 8.0, 7: 8.0}

    def tapw(f, t):
        # [64, 128] weight block: plane u^f/PSC[f], tap t
        return (wm[f][:, cidx * NTAPS + t].T * PSC[f]).astype(np.float32)

    # chunk A = [x | x^2]: 9 taps
    wa = np.zeros((128, NTAPS, C_OUT), np.float32)
    for t in range(NTAPS):
        wa[0:64, t] = tapw(1, t)
        wa[64:128, t] = tapw(2, t)
    # chunks B = [x^3|x^4], C = [x^5|x^6]: 9 taps each
    wl = np.zeros((128, 2 * NTAPS + ND, C_OUT), np.float32)
    for t in range(NTAPS):
        wl[0:64, t] = tapw(3, t)
        wl[64:128, t] = tapw(4, t)
        wl[0:64, NTAPS + t] = tapw(5, t)
        wl[64:128, NTAPS + t] = tapw(6, t)
    # chunk D = [x^7 | x^7>>1]: 6 passes (dh, o); lower = tap (dh, o-1),
    # upper = tap (dh, 1) on o==1 passes only
    for p in range(ND):
        dh, o = p // 2 - 1, p % 2
        t_lo = (dh + 1) * K + (o - 1 + 1)
        wl[0:64, 2 * NTAPS + p] = tapw(7, t_lo)
        if o == 1:
            wl[64:128, 2 * NTAPS + p] = tapw(7, (dh + 1) * K + 2)
    return (
        wa.reshape(128, NTAPS * 128).astype(np.float16),
        wl.reshape(128, (2 * NTAPS + ND) * 128).astype(np.float16),
        bias.astype(np.float32),
    )


def _prep_in_maps(x, w_b, w_s, c):
    wa, wl, bias = _host_prep(x, w_b, w_s, c)
    xi = np.asarray(x, np.float16)
    xp = np.zeros((B, C_IN, HP, WP), np.float16)
    xp[:, :, 1 : 1 + H, 1 : 1 + W] = xi
    xp = xp.reshape(B, C_IN, PADN)
    in_maps = []
    for i in range(B):
        in_maps.append({"xl": xp[i], "xu": xp[i], "wa": wa, "w": wl})
    return in_maps, bias


def kernel(x, w_b, w_s, c):
    if "nc" not in _CACHE:
        _CACHE["nc"] = _build_program()
    nc = _CACHE["nc"]

    in_maps, bias = _prep_in_maps(x, w_b, w_s, c)
    res = run_bass_kernel_spmd(nc, in_maps, core_ids=list(range(B)))
    out = np.stack(
        [res.results[i]["out"].astype(np.float32) for i in range(B)], axis=0
    )
    out *= OUT_SCALE
    out += bias[None, :, None]
    return out.reshape(B, C_OUT, H, W)
